# revision 4
# baseline (speedup 1.0000x reference)
"""Linear attention (elu+1 feature map) Bass/Tile kernel for Trainium2.

Full inputs: queries/keys/values [N=8, L/S=8192, H=8, D=64] fp32.
Sharding: data-parallel over N across the 8 NeuronCores (batch i -> core i).
Host<->device I/O and on-device compute are fp16 (inputs quantized on the
host, output upcast on the host) -- this halves the dominant per-execution
I/O cost and doubles matmul/DVE throughput; all matmul accumulation stays
fp32 in PSUM.  The /S, *S factors in the reference cancel exactly.

Math per (n, h):
  Q' = elu(Q)+1, K' = elu(K)+1
  KV[d, v] = sum_s K'[s, d] V[s, v];  Ksum[d] = sum_s K'[s, d]
  out[l, v] = (Q'[l, :] @ KV[:, v]) / (Q'[l, :] @ Ksum + eps)

Kernel structure per core:
  Phase 1 (stream K, V in 1024-row chunks): feature-map K as
    t=max(-x,0) [DVE/any]; e=exp(-t) [ACT, the only ACT function -> its
    table loads once]; k'=max(x,0)+e [DVE].  V is spread into per-head-pair
    blocks [V_h2j | V_h2j+1 | ones] of width 130 so each pair accumulates
    [KV | Ksum] with a single matmul stream per PSUM bank:
    lhsT=K'_pair [128s, 128d], rhs=[128s, 129].
  Phase 2 (stream Q in 1024-row chunks): hardware DMA-transpose (fp16 xbar)
    loads Q^T per 128-column pair group -- no PE transposes -- then the same
    feature map, then per 128-row l-block: 4 matmuls
    (rhs=w2a[g] [128, 130], block-diagonal [KV pair | Ksum pair]) into two
    [128, 2, 130]
    PSUM banks; the Ksum columns ride along in the same matmul (weights
    [128, 130] = [KV pair | Ksum pair]), so numerator and denominator come
    from one matmul stream per pair.  The epilogue reciprocals the two
    denominator columns and broadcast-multiplies (stride-0 AP), writing
    fp16 output.  (The reference's +1e-6 eps is dropped: den = Q'.Ksum is
    > ~1e4 for this feature map, so eps is far below fp16 output ulp.)
"""

import functools
import sys

sys.path.insert(0, "/opt/trn_rl_repo")

import numpy as np

import concourse.bass as bass
import concourse.mybir as mybir
import concourse.tile as tile
from concourse import bacc
from concourse.bass_utils import run_bass_kernel_spmd

N, L, S, H, D = 8, 8192, 8192, 8, 64
P = 128
HD = H * D
FP32 = mybir.dt.float32
FP16 = mybir.dt.float16
AF = mybir.ActivationFunctionType
OP = mybir.AluOpType

SC = 1024  # rows per chunk in both phases


def _feature_map(nc, tpool, epool, x_ap, out_ap, shape, tag):
    """out = elu(x)+1 = max(x,0) + exp(-max(-x,0)).

    t = max(-x, 0) [any engine]; e = exp(-t) [ACT]; out = max(x, 0) + e
    [DVE].  ACT only ever computes Exp in this kernel, so its function
    table is loaded once.  Every op waits on <=2 distinct upstream
    semaphores.
    """
    t = tpool.tile(shape, FP16, name=f"fm_t_{tag}", tag=f"fm_t_{tag}")
    e = epool.tile(shape, FP16, name=f"fm_e_{tag}", tag=f"fm_e_{tag}")
    nc.any.tensor_scalar(t, x_ap, -1.0, 0.0, OP.mult, OP.max)
    nc.scalar.activation(e, t, AF.Exp, scale=-1.0)
    nc.vector.scalar_tensor_tensor(
        out_ap, in0=x_ap, scalar=0.0, in1=e, op0=OP.max, op1=OP.add
    )


def build_kernel(L_=L, S_=S, repeat=1):
    nc = bacc.Bacc(trn_type="TRN2")
    q_d = nc.dram_tensor("queries", [L_, HD], FP16, kind="ExternalInput")
    k_d = nc.dram_tensor("keys", [S_, HD], FP16, kind="ExternalInput")
    v_d = nc.dram_tensor("values", [S_, HD], FP16, kind="ExternalInput")
    o_d = nc.dram_tensor("out", [L_, HD], FP16, kind="ExternalOutput")

    n_kc = S_ // SC
    n_qc = L_ // SC
    nsub = SC // P  # 128-row sub-blocks per chunk

    with tile.TileContext(nc) as tc:
        with (
            tc.tile_pool(name="kdma", bufs=3) as kdma,
            tc.tile_pool(name="vdma", bufs=3) as vdma,
            tc.tile_pool(name="vsp", bufs=2) as vspp,
            tc.tile_pool(name="fmt", bufs=2) as fmt,
            tc.tile_pool(name="fme", bufs=2) as fme,
            tc.tile_pool(name="kp", bufs=2) as kpp,
            tc.tile_pool(name="w2", bufs=1) as w2p,
            tc.tile_pool(name="qdma", bufs=3) as qdma,
            tc.tile_pool(name="qp", bufs=2) as qpp,
            tc.tile_pool(name="zr", bufs=2) as zrp,
            tc.tile_pool(name="outp", bufs=2) as outp,
            tc.tile_pool(name="kvps", bufs=1, space="PSUM") as kvps,
            tc.tile_pool(name="nump", bufs=2, space="PSUM") as nump,
        ):
            for rep in range(repeat):
                # ---- Phase 1: accumulate [KV | Ksum] per head pair ----
                kv_ps = [
                    kvps.tile([P, 129], FP32, name=f"kv{j}", tag=f"kv{j}")
                    for j in range(4)
                ]
                for cc in range(n_kc):
                    r0 = cc * SC
                    ktile = kdma.tile([P, nsub, HD], FP16, name="ktile", tag="ktile")
                    nc.sync.dma_start(
                        ktile,
                        k_d[r0 : r0 + SC, :].rearrange("(sub p) f -> p sub f", p=P),
                    )
                    vtile = vdma.tile([P, nsub, HD], FP16, name="vtile", tag="vtile")
                    nc.sync.dma_start(
                        vtile,
                        v_d[r0 : r0 + SC, :].rearrange("(sub p) f -> p sub f", p=P),
                    )
                    # spread V into per-pair blocks [V_pair | ones] (width 130)
                    vsp = vspp.tile([P, nsub, 4, 130], FP16, name="vsp", tag="vsp")
                    nc.vector.memset(vsp[:, :, :, 128:129], 1.0)
                    nc.any.tensor_copy(
                        vsp[:, :, :, 0:128],
                        vtile.rearrange("p sub (j e) -> p sub j e", j=4),
                    )
                    kp = kpp.tile([P, nsub, HD], FP16, name="kp", tag="kp")
                    _feature_map(nc, fmt, fme, ktile, kp, [P, nsub, HD], "k")
                    for sub in range(nsub):
                        for j in range(4):
                            nc.tensor.matmul(
                                kv_ps[j],
                                lhsT=kp[:, sub, j * P : (j + 1) * P],
                                rhs=vsp[:, sub, j, 0:129],
                                start=(cc == 0 and sub == 0),
                                stop=(cc == n_kc - 1 and sub == nsub - 1),
                            )

                # ---- Phase 1.5: block-diagonal fp16 weights ----
                # w2a[g] [128, 130]: KV_2g in rows/cols 0:64, KV_2g+1 in
                # rows/cols 64:128, Ksum_2g in rows 0:64 col 128, Ksum_2g+1
                # in rows 64:128 col 129, zeros elsewhere.
                w2a = [
                    w2p.tile([P, 130], FP16, name=f"w2a{g}", tag=f"w2a{g}")
                    for g in range(4)
                ]
                for g in range(4):
                    nc.vector.memset(w2a[g], 0.0)
                    nc.vector.tensor_copy(w2a[g][0:64, 0:64], kv_ps[g][0:64, 0:64])
                    nc.vector.tensor_copy(
                        w2a[g][64:128, 64:128], kv_ps[g][64:128, 64:128]
                    )
                    nc.vector.tensor_copy(
                        w2a[g][0:64, 128:129], kv_ps[g][0:64, 128:129]
                    )
                    nc.vector.tensor_copy(
                        w2a[g][64:128, 129:130], kv_ps[g][64:128, 128:129]
                    )

                # ---- Phase 2: stream Q ----
                for cc in range(n_qc):
                    r0 = cc * SC
                    qraw = qdma.tile([P, 4, SC], FP16, name="qraw", tag="qraw")
                    for g in range(4):
                        nc.sync.dma_start(
                            qraw[:, g, :],
                            q_d[r0 : r0 + SC, g * P : (g + 1) * P],
                            transpose=True,
                        )
                    qp = qpp.tile([P, 4, SC], FP16, name="qp", tag="qp")
                    _feature_map(nc, fmt, fme, qraw, qp, [P, 4, SC], "q")

                    otile = outp.tile([P, nsub, H, D], FP16, name="otile", tag="otile")
                    for sub in range(nsub):
                        for g2 in range(2):
                            p2 = nump.tile(
                                [P, 2, 130], FP32, name=f"p2_{g2}", tag=f"p2_{g2}"
                            )
                            for gg in range(2):
                                g = 2 * g2 + gg
                                nc.tensor.matmul(
                                    p2[:, gg, :],
                                    lhsT=qp[:, g, sub * P : (sub + 1) * P],
                                    rhs=w2a[g],
                                    start=True,
                                    stop=True,
                                )
                            zr = zrp.tile(
                                [P, 2, 2], FP32, name=f"zr{g2}", tag=f"zr{g2}"
                            )
                            nc.vector.reciprocal(zr, p2[:, :, 128:130])
                            nc.any.tensor_tensor(
                                otile[:, sub, 4 * g2 : 4 * g2 + 4].rearrange(
                                    "p (pr hh) d -> p pr hh d", pr=2
                                ),
                                p2[:, :, 0:128].rearrange(
                                    "p pr (hh d) -> p pr hh d", hh=2
                                ),
                                zr.unsqueeze(3).broadcast_to([P, 2, 2, D]),
                                OP.mult,
                            )
                    nc.sync.dma_start(
                        o_d[r0 : r0 + SC, :].rearrange("(sub p) f -> p sub f", p=P),
                        otile.rearrange("p sub h d -> p sub (h d)"),
                    )
    nc.compile()
    return nc


@functools.lru_cache(maxsize=None)
def _cached_nc(L_, S_):
    return build_kernel(L_, S_)


def kernel(queries: np.ndarray, keys: np.ndarray, values: np.ndarray) -> np.ndarray:
    n, l_, h, d = queries.shape
    s_ = keys.shape[1]
    nc = _cached_nc(l_, s_)
    in_maps = [
        {
            "queries": queries[i].reshape(l_, h * d).astype(np.float16),
            "keys": keys[i].reshape(s_, h * d).astype(np.float16),
            "values": values[i].reshape(s_, h * d).astype(np.float16),
        }
        for i in range(n)
    ]
    res = run_bass_kernel_spmd(nc, in_maps, core_ids=list(range(n)))
    out = np.empty((n, l_, h, d), np.float32)
    for i in range(n):
        out[i] = res.results[i]["out"].reshape(l_, h, d)
    return out


if __name__ == "__main__":
    nc = build_kernel()
    print("build ok")
